# revision 5
# baseline (speedup 1.0000x reference)
"""Trainium2 Bass kernel for the pairwise concordance-index loss.

reference:
    loss = sum_{i<j, f_i=f_j=1} relu((p_i-p_j)(t_i-t_j)) / 100 / n_pairs

Math:
  Only flagged (f=1) entries contribute, so the host first COMPACTS the
  arrays to the n1 flagged entries (padded with zeros to NB*128), which
  shrinks the pairwise matrix from B^2 to ~(0.7B)^2.
  M[i,j] = (p_i-p_j)(t_i-t_j) = A^T B, rank 4:
      A = [u, 1, p, t],  B = [1, u, -t, -p],  u = p*t   (zeros in padding)
  sum relu(M) = 0.5*(sum M + sum |M|); sum M has an O(n) closed form done
  on the host in fp64; sum |M| is the O(n^2) part done on device.

Device decomposition (8 cores, identical program, data-sharded):
  NB row-blocks of 128 rows; core k owns NBC=NB/8 blocks as a quad gang
  (4 blocks) + duo gang (NBC-4). Each block processes cyclic column
  offsets e=0..NB/2 (cols 128(a+e) mod NB*128): e=1..NB/2-1 at weight 1;
  e=0 / e=NB/2 at weight 0.5 via 0.5-pre-scaled slab appendices
  (host-side), so all device sums have uniform weight.

Device structure (raw Bass, hand-rolled semaphores — no TileContext):
  PE: per 2-bank PSUM tile, 2-4 K=4 bf16 matmuls packed into disjoint
  32-row PE groups via tile_position (concurrent matmuls always target
  distinct PSUM banks). Each tile is consumed by ONE fused abs-row-sum
  job on the DVE (tensor_reduce(apply_absolute_value)) or the ScalarE
  (activation(Abs, accum_out)), greedily load-balanced across the two.
  PSUM is an 8-bank ring of four 2-bank buffers; the PE waits on the
  consumer semaphore before reusing a buffer.

  Input DMAs use one counting semaphore per HWDGE queue (sync, scalar)
  — completions within a queue are in order, so a tile waits for a
  cumulative count instead of per-chunk semaphores.  The 'a' (weights)
  transfer goes first on the sync queue; each B replica is split at
  CUT so tiles that only touch the first half can start early.  Tiles
  are issued in data-arrival order.

  The output DMA (on the idle sync engine) carries no semaphore and is
  not waited on: the NEFF postamble that follows (walrus's ~7us
  semaphore-reset storm) far exceeds the DMA drain time, so the store
  completes well before the runtime reads outputs.  Set KEEP_OUT_WAIT=1
  to restore the explicit completion wait.
"""

import numpy as np

B = 8192
P = 128
NCORE = 8
CUT = 2560

_cache = {}


def _plan(n1):
    """Compile-time plan derived from the flagged count."""
    nb = max(1, -(-n1 // P))        # 128-row blocks needed
    nb = -(-nb // NCORE) * NCORE    # multiple of 8 (even)
    nbc = nb // NCORE               # blocks per core
    eh = nb // 2                    # antipodal offset (weight 1/2)
    mainw = P * (nbc - 1 + eh - 1)  # shared slab for e=1..eh-1
    tailw = nbc * 256               # per-block [e0/2 | e_h/2] appendix
    gangs = [4] * (nbc // 4) + ([nbc % 4] if nbc % 4 else [])
    w = P * (eh - 1)                # main cols per block
    nfull, rem = w // 512, w % 512
    return dict(nb=nb, nbc=nbc, eh=eh, mainw=mainw, tailw=tailw,
                bcols=mainw + tailw, gangs=gangs, nfull=nfull, rem=rem)


# DMA queue orders: (chunk name -> (queue, arrival index, threshold)).
# sync queue: a, b0h1, b2h1, b0h2, b2h2 ; scalar queue: b1h1, b3h1, b1h2, b3h2
_SYNC_ORDER = ["a", "b0h1", "b2h1", "b0h2", "b2h2"]
_SCAL_ORDER = ["b1h1", "b3h1", "b1h2", "b3h2"]


def _chunk_info():
    info = {}
    for i, c in enumerate(_SYNC_ORDER):
        info[c] = ("s", i + 0.0, 16 * (i + 1))
    for i, c in enumerate(_SCAL_ORDER):
        info[c] = ("c", i + 0.5, 16 * (i + 1))
    return info


def _layout(plan):
    """Ordered tile descriptors: matmul lists, reduce specs, DMA gates."""
    gangs, rem, mainw = plan["gangs"], plan["rem"], plan["mainw"]
    cinfo = _chunk_info()
    tiles = []
    for g, sz in enumerate(gangs):
        off = sum(gangs[:g])
        npair = (sz + 1) // 2
        for pi in range(npair):
            qs = [q for q in (2 * pi, 2 * pi + 1) if q < sz]
            for s in range(plan["nfull"]):
                mms = [(q, P * (off + q) + 512 * s, 512, qi, 0)
                       for qi, q in enumerate(qs)]
                tiles.append(dict(acol=P * g, mms=mms, kind="main"))
            if rem:
                mms = [(q, P * (off + q) + 512 * plan["nfull"], rem, qi, 0)
                       for qi, q in enumerate(qs)]
                tiles.append(dict(acol=P * g, mms=mms, kind="rem"))
            # tail: per block two N=128 half-weight columns, one bank per
            # block; two waves so concurrent matmuls never share a bank
            mms = []
            for wave in range(2):
                for qi, q in enumerate(qs):
                    coff = mainw + 256 * (off + q) + 128 * wave
                    mms.append((q, coff, 128, qi, 128 * wave))
            tiles.append(dict(acol=P * g, mms=mms, kind="tail"))
    # per-tile chunk requirements -> (queue, threshold); arrival rank
    for t in tiles:
        need = {"a"}
        for (q, coff, n, _b, _c) in t["mms"]:
            if coff < CUT:
                need.add(f"b{q}h1")
            if coff + n > CUT:
                need.add(f"b{q}h2")
        t["nbank"] = len(set(b for (_, _, _, b, _) in t["mms"]))
        t["width"] = max(c + n for (_, _, n, _, c) in t["mms"])
        t["fd"] = sum(n for (_, _, n, _, _) in t["mms"])
        t["gates"] = {}
        rank = 0.0
        for c in need:
            qn, arr, thr = cinfo[c]
            rank = max(rank, arr)
            t["gates"][qn] = max(t["gates"].get(qn, 0), thr)
        t["rank"] = rank
    # issue order: by data arrival, stable within rank
    tiles.sort(key=lambda t: t["rank"])
    # consumer assignment: greedy balance of projected finish times.
    # V: (120 + fd)/0.96 ns ; A: (172 + fd)/1.2 + 307 ns  (read-accum)
    tv = ta = 0.0
    for t in tiles:
        cv = (120 + t["fd"]) / 0.96
        ca = (172 + t["fd"]) / 1.2 + 307.0
        if tv + cv <= ta + ca:
            t["eng"] = "V"
            tv += cv
        else:
            t["eng"] = "A"
            ta += ca
    return tiles


def _build(plan):
    """Build + compile the raw Bass module (once per plan)."""
    import os
    import concourse.bacc as bacc
    import concourse.mybir as mybir

    f32 = mybir.dt.float32
    bf16 = mybir.dt.bfloat16
    nc = bacc.Bacc("TRN2", target_bir_lowering=False, debug=False,
                   num_devices=NCORE)

    gangs = plan["gangs"]
    bcols = plan["bcols"]
    awidth = P * len(gangs)
    tiles = _layout(plan)
    njobs = len(tiles)
    nV = sum(1 for t in tiles if t["eng"] == "V")
    nA = njobs - nV

    a_dram = nc.dram_tensor("a_rows", [P, awidth], bf16, kind="ExternalInput")
    b_dram = nc.dram_tensor("b_cols", [4, bcols], bf16, kind="ExternalInput")
    acc_dram = nc.dram_tensor("acc", [P, njobs], f32, kind="ExternalOutput")

    a_sb = nc.alloc_sbuf_tensor("a_sb", [P, awidth], bf16)
    b_sb = nc.alloc_sbuf_tensor("b_sb", [P, bcols], bf16)
    acc_sb = nc.alloc_sbuf_tensor("acc_sb", [P, njobs], f32)
    ps = nc.alloc_psum_tensor("ps", [P, 8, 512], f32)

    sem_mm = nc.alloc_semaphore("sem_mm")    # tiles filled by PE
    sem_v = nc.alloc_semaphore("sem_v")      # DVE jobs done
    sem_a = nc.alloc_semaphore("sem_a")      # ACT jobs done
    sem_s = nc.alloc_semaphore("sem_in_s")   # sync-queue input counter
    sem_c = nc.alloc_semaphore("sem_in_c")   # scalar-queue input counter
    qsem = {"s": sem_s, "c": sem_c}
    keep_wait = bool(os.environ.get("KEEP_OUT_WAIT"))
    sem_out = nc.alloc_semaphore("sem_out")

    # per-tile bookkeeping for sync
    jobidx = {}
    counts = {"V": 0, "A": 0}
    for i, t in enumerate(tiles):
        counts[t["eng"]] += 1
        jobidx[i] = counts[t["eng"]]  # 1-based within its engine

    half2 = [(0, CUT), (CUT, bcols)]

    with nc.Block("k") as blk:

        @blk.sync
        def _(eng):
            # weights first (every tile needs them), then replicas 0/2
            eng.dma_start(a_sb.ap()[:, :], a_dram.ap()[:, :]).then_inc(
                sem_s, 16)
            for h, (c0, c1) in enumerate(half2):
                for q in (0, 2):
                    eng.dma_start(b_sb.ap()[32 * q:32 * q + 4, c0:c1],
                                  b_dram.ap()[:, c0:c1]).then_inc(sem_s, 16)

        @blk.scalar
        def _(eng):
            for h, (c0, c1) in enumerate(half2):
                for q in (1, 3):
                    eng.dma_start(b_sb.ap()[32 * q:32 * q + 4, c0:c1],
                                  b_dram.ap()[:, c0:c1]).then_inc(sem_c, 16)
            for i, t in enumerate(tiles):
                if t["eng"] != "A":
                    continue
                buf = i % 4
                eng.wait_ge(sem_mm, i + 1)
                red = ps.ap()[:, 2 * buf:2 * buf + t["nbank"], 0:t["width"]]
                eng.activation(
                    red, red,
                    mybir.ActivationFunctionType.Abs,
                    accum_out=acc_sb.ap()[:, i:i + 1],
                ).then_inc(sem_a, 1)
            # output from THIS engine: its own accumulator-column writes
            # (ACTIVATE -> READ_ACCUMULATOR) are ordered by program order;
            # DVE columns are fenced by sem_v (tensor_reduce incs after its
            # write).  No completion semaphore: the NEFF postamble outlasts
            # the DMA drain.
            eng.wait_ge(sem_v, nV)
            eng.dma_start(acc_dram.ap()[:, :], acc_sb.ap()[:, :]).then_inc(
                sem_out, 16)
            if keep_wait:
                eng.wait_ge(sem_out, 16)

        @blk.tensor
        def _(eng):
            seen = {"s": 0, "c": 0}
            for i, t in enumerate(tiles):
                for qn in ("s", "c"):
                    thr = t["gates"].get(qn, 0)
                    if thr > seen[qn]:
                        seen[qn] = thr
                        eng.wait_ge(qsem[qn], thr)
                if i >= 4:
                    p = i - 4  # previous occupant of this 2-bank buffer
                    eng.wait_ge(sem_v if tiles[p]["eng"] == "V" else sem_a,
                                jobidx[p])
                buf = i % 4
                last = len(t["mms"]) - 1
                for j, (q, coff, n, bank, c0) in enumerate(t["mms"]):
                    ins = nc.tensor.matmul(
                        ps.ap()[:, 2 * buf + bank, c0:c0 + n],
                        a_sb.ap()[32 * q:32 * q + 4, t["acol"]:t["acol"] + P],
                        b_sb.ap()[32 * q:32 * q + 4, coff:coff + n],
                        start=True,
                        stop=True,
                        tile_position=(32 * q, 0),
                    )
                    if j == last:
                        ins.then_inc(sem_mm, 1)

        @blk.vector
        def _(eng):
            for i, t in enumerate(tiles):
                if t["eng"] != "V":
                    continue
                buf = i % 4
                eng.wait_ge(sem_mm, i + 1)
                eng.tensor_reduce(
                    acc_sb.ap()[:, i:i + 1],
                    ps.ap()[:, 2 * buf:2 * buf + t["nbank"], 0:t["width"]],
                    axis=mybir.AxisListType.XY, op=mybir.AluOpType.add,
                    apply_absolute_value=True,
                ).then_inc(sem_v, 1)

    nc.compile()
    return nc


def _get_nc(plan):
    key = ("nc", plan["nb"])
    if key not in _cache:
        _cache[key] = _build(plan)
    return _cache[key]


def _prepare(pred, gt, ift, imf):
    """Compact + pad + build per-core input maps."""
    import ml_dtypes

    p_full = np.asarray(pred).astype(np.float32)
    gt = np.asarray(gt).astype(np.float32)
    t_full = gt[:, ift]
    f_full = gt[:, imf] == 1
    idx = np.flatnonzero(f_full)
    n1 = len(idx)

    plan = _plan(n1)
    npad = plan["nb"] * P
    p = np.zeros(npad, np.float32)
    t = np.zeros(npad, np.float32)
    w = np.zeros(npad, np.float32)
    p[:n1] = p_full[idx]
    t[:n1] = t_full[idx]
    w[:n1] = 1.0
    u = p * t

    # compaction makes the flags trivial: real entries are all flagged,
    # padded entries are exactly zero in every factor.
    A = np.ascontiguousarray(
        np.stack([u, w, p, t]).astype(ml_dtypes.bfloat16)
    )
    Bm = np.ascontiguousarray(
        np.stack([w, u, -t, -p]).astype(ml_dtypes.bfloat16)
    )
    Bh = Bm * np.asarray(0.5, dtype=ml_dtypes.bfloat16)  # exact halving

    nbc, eh, mainw = plan["nbc"], plan["eh"], plan["mainw"]
    gangs = plan["gangs"]
    awidth = P * len(gangs)
    in_maps = []
    for k in range(NCORE):
        a_rows = np.zeros((P, awidth), dtype=ml_dtypes.bfloat16)
        for g, sz in enumerate(gangs):
            off = sum(gangs[:g])
            for q in range(sz):
                blk = nbc * k + off + q
                a_rows[32 * q:32 * q + 4, P * g:P * g + P] = \
                    A[:, P * blk:P * blk + P]

        b_colsk = np.empty((4, plan["bcols"]), dtype=ml_dtypes.bfloat16)
        cols = (P * (nbc * k + 1) + np.arange(mainw)) % npad
        b_colsk[:, 0:mainw] = Bm[:, cols]
        for a in range(nbc):
            c0 = (P * (nbc * k + a) + np.arange(P)) % npad
            ch = (P * (nbc * k + a + eh) + np.arange(P)) % npad
            base = mainw + 256 * a
            b_colsk[:, base:base + P] = Bh[:, c0]
            b_colsk[:, base + P:base + 256] = Bh[:, ch]
        in_maps.append(
            {"a_rows": a_rows, "b_cols": np.ascontiguousarray(b_colsk)}
        )
    return in_maps, A, Bm, n1, plan


def kernel(pred, gt, gt_fracTime, gt_ifMOF):
    from concourse import bass_utils

    ift = int(np.asarray(gt_fracTime))
    imf = int(np.asarray(gt_ifMOF))

    in_maps, A, Bm, n1, plan = _prepare(pred, gt, ift, imf)
    nc = _get_nc(plan)
    res = bass_utils.run_bass_kernel_spmd(nc, in_maps,
                                          core_ids=list(range(NCORE)))

    # T = sum_{i<j} |M| (all device accumulator columns are weight 1)
    T = 0.0
    for r in res.results:
        T += r["acc"].astype(np.float64).sum()

    # host closed form in fp64 over the same bf16 values the device used:
    # sum_{i<j} M = (sum_{i,j} M - sum_diag M) / 2
    A64 = A.astype(np.float64)
    B64 = Bm.astype(np.float64)
    S_all = (A64.sum(axis=1) * B64.sum(axis=1)).sum()
    D_diag = (A64 * B64).sum()
    S_half = (S_all - D_diag) / 2.0

    n_pairs = (float(n1) * float(n1) - float(n1)) / 2.0

    loss = 0.5 * (S_half + T) / 100.0 / n_pairs
    return np.asarray(np.float32(loss))


# revision 10
# speedup vs baseline: 1.1174x; 1.1174x over previous
"""Trainium2 Bass kernel for the pairwise concordance-index loss.

reference:
    loss = sum_{i<j, f_i=f_j=1} relu((p_i-p_j)(t_i-t_j)) / 100 / n_pairs

Math:
  Only flagged (f=1) entries contribute, so the host first COMPACTS the
  arrays to the n1 flagged entries (padded with zeros to NB*128), which
  shrinks the pairwise matrix from B^2 to ~(0.7B)^2.
  M[i,j] = (p_i-p_j)(t_i-t_j) = A^T B, rank 4:
      A = [u, 1, p, t],  B = [1, u, -t, -p],  u = p*t   (zeros in padding)
  sum relu(M) = 0.5*(sum M + sum |M|); sum M has an O(n) closed form done
  on the host in fp64; sum |M| is the O(n^2) part done on device.

Device decomposition (8 cores, identical program, data-sharded):
  NB row-blocks of 128 rows; core k owns NBC=NB/8 blocks as a quad gang
  (4 blocks) + duo gang (NBC-4). Each block processes cyclic column
  offsets e=0..NB/2 (cols 128(a+e) mod NB*128): e=1..NB/2-1 at weight 1;
  e=0 / e=NB/2 at weight 0.5 via 0.5-pre-scaled slab appendices
  (host-side), so all device sums have uniform weight.

Device structure (raw Bass, hand-rolled semaphores — no TileContext):
  PE: per 2-bank PSUM tile, 2-4 K=4 bf16 matmuls packed into disjoint
  32-row PE groups via tile_position (concurrent matmuls always target
  distinct PSUM banks). Each tile is consumed by ONE fused abs-row-sum
  job on the DVE (tensor_reduce(apply_absolute_value)) or the ScalarE
  (activation(Abs, accum_out)), greedily load-balanced across the two.
  PSUM is an 8-bank ring of four 2-bank buffers; the PE waits on the
  consumer semaphore before reusing a buffer.

  Input DMAs use one counting semaphore per HWDGE queue (sync, scalar)
  — completions within a queue are in order, so a tile waits for a
  cumulative count instead of per-chunk semaphores.  The 'a' (weights)
  transfer goes first on the sync queue; each B replica is split at
  CUT so tiles that only touch the first half can start early.  Tiles
  are issued in data-arrival order.

  The output DMA (on the idle sync engine) carries no semaphore and is
  not waited on: the NEFF postamble that follows (walrus's ~7us
  semaphore-reset storm) far exceeds the DMA drain time, so the store
  completes well before the runtime reads outputs.  Set KEEP_OUT_WAIT=1
  to restore the explicit completion wait.
"""

import numpy as np

B = 8192
P = 128
NCORE = 8
CUT = 2560

_cache = {}


def _plan(n1):
    """Compile-time plan derived from the flagged count."""
    nb = max(1, -(-n1 // P))        # 128-row blocks needed
    nb = -(-nb // NCORE) * NCORE    # multiple of 8 (even)
    nbc = nb // NCORE               # blocks per core
    eh = nb // 2                    # antipodal offset (weight 1/2)
    mainw = P * (nbc - 1 + eh - 1)  # shared slab for e=1..eh-1
    tailw = nbc * 256               # per-block [e0/2 | e_h/2] appendix
    gangs = [4] * (nbc // 4) + ([nbc % 4] if nbc % 4 else [])
    w = P * (eh - 1)                # main cols per block
    nfull, rem = w // 512, w % 512
    return dict(nb=nb, nbc=nbc, eh=eh, mainw=mainw, tailw=tailw,
                bcols=mainw + tailw, gangs=gangs, nfull=nfull, rem=rem)


# DMA queue orders.  Completions within a HWDGE queue interleave across
# physical DMA engines, so each chunk gets its OWN semaphore (wait >= 16
# means that chunk fully landed); the order below only sets arrival time.
_SYNC_ORDER = ["a", "b0h1", "b2h1", "b0h2", "b2h2", "b3h2"]
_SCAL_ORDER = ["b1h1", "b3h1", "b1h2"]


def _chunk_info():
    info = {}
    for i, c in enumerate(_SYNC_ORDER):
        info[c] = ("s", i + 0.0)
    for i, c in enumerate(_SCAL_ORDER):
        info[c] = ("c", i + 0.5)
    return info


def _layout(plan):
    """Ordered tile descriptors: matmul lists, reduce specs, DMA gates."""
    gangs, rem, mainw = plan["gangs"], plan["rem"], plan["mainw"]
    cinfo = _chunk_info()
    tiles = []
    for g, sz in enumerate(gangs):
        off = sum(gangs[:g])
        npair = (sz + 1) // 2
        for pi in range(npair):
            qs = [q for q in (2 * pi, 2 * pi + 1) if q < sz]
            for s in range(plan["nfull"]):
                mms = [(q, P * (off + q) + 512 * s, 512, qi, 0)
                       for qi, q in enumerate(qs)]
                tiles.append(dict(acol=P * g, mms=mms, kind="main"))
            if rem:
                mms = [(q, P * (off + q) + 512 * plan["nfull"], rem, qi, 0)
                       for qi, q in enumerate(qs)]
                tiles.append(dict(acol=P * g, mms=mms, kind="rem"))
            # tail: per block two N=128 half-weight columns, one bank per
            # block; two waves so concurrent matmuls never share a bank
            mms = []
            for wave in range(2):
                for qi, q in enumerate(qs):
                    coff = mainw + 256 * (off + q) + 128 * wave
                    mms.append((q, coff, 128, qi, 128 * wave))
            tiles.append(dict(acol=P * g, mms=mms, kind="tail"))
    # per-tile chunk requirements -> (queue, threshold); arrival rank
    for t in tiles:
        need = {"a"}
        for (q, coff, n, _b, _c) in t["mms"]:
            if coff < CUT:
                need.add(f"b{q}h1")
            if coff + n > CUT:
                need.add(f"b{q}h2")
        t["nbank"] = len(set(b for (_, _, _, b, _) in t["mms"]))
        t["width"] = max(c + n for (_, _, n, _, c) in t["mms"])
        t["fd"] = sum(n for (_, _, n, _, _) in t["mms"])
        t["chunks"] = need
        t["rank"] = max(cinfo[c][1] for c in need)
    # issue order: by data arrival, stable within rank
    tiles.sort(key=lambda t: t["rank"])
    # consumer assignment: greedy balance of projected finish times.
    # V: (120 + fd)/0.96 ns ; A: (172 + fd)/1.2 + 307 ns  (read-accum)
    tv = ta = 0.0
    for t in tiles:
        cv = (120 + t["fd"]) / 0.96
        ca = (172 + t["fd"]) / 1.2 + 307.0
        if tv + cv <= ta + ca:
            t["eng"] = "V"
            tv += cv
        else:
            t["eng"] = "A"
            ta += ca
    return tiles


def _build(plan):
    """Build + compile the raw Bass module (once per plan)."""
    import os
    import concourse.bacc as bacc
    import concourse.mybir as mybir

    f32 = mybir.dt.float32
    bf16 = mybir.dt.bfloat16
    nc = bacc.Bacc("TRN2", target_bir_lowering=False, debug=False,
                   num_devices=NCORE)

    gangs = plan["gangs"]
    bcols = plan["bcols"]
    awidth = P * len(gangs)
    tiles = _layout(plan)
    njobs = len(tiles)
    nV = sum(1 for t in tiles if t["eng"] == "V")
    nA = njobs - nV

    a_dram = nc.dram_tensor("a_rows", [P, awidth], bf16, kind="ExternalInput")
    b_dram = nc.dram_tensor("b_cols", [4, bcols], bf16, kind="ExternalInput")
    acc_dram = nc.dram_tensor("acc", [P, njobs], f32, kind="ExternalOutput")

    a_sb = nc.alloc_sbuf_tensor("a_sb", [P, awidth], bf16)
    b_sb = nc.alloc_sbuf_tensor("b_sb", [P, bcols], bf16)
    acc_sb = nc.alloc_sbuf_tensor("acc_sb", [P, njobs], f32)
    ps = nc.alloc_psum_tensor("ps", [P, 8, 512], f32)

    sem_mm = nc.alloc_semaphore("sem_mm")    # tiles filled by PE
    sem_v = nc.alloc_semaphore("sem_v")      # DVE jobs done
    sem_a = nc.alloc_semaphore("sem_a")      # ACT jobs done
    # one sem per input chunk: completions interleave within a queue
    chunk_order = _SYNC_ORDER + _SCAL_ORDER
    sem_chunk = {c: nc.alloc_semaphore(f"sem_in_{c}") for c in chunk_order}
    keep_wait = bool(os.environ.get("KEEP_OUT_WAIT"))
    sem_out = nc.alloc_semaphore("sem_out")

    # per-tile bookkeeping for sync
    jobidx = {}
    counts = {"V": 0, "A": 0}
    for i, t in enumerate(tiles):
        counts[t["eng"]] += 1
        jobidx[i] = counts[t["eng"]]  # 1-based within its engine

    def emit_chunk_dma(eng, c):
        if c == "a":
            eng.dma_start(a_sb.ap()[:, :], a_dram.ap()[:, :]).then_inc(
                sem_chunk[c], 16)
            return
        q = int(c[1])
        c0, c1 = (0, CUT) if c[3] == "1" else (CUT, bcols)
        eng.dma_start(b_sb.ap()[32 * q:32 * q + 4, c0:c1],
                      b_dram.ap()[:, c0:c1]).then_inc(sem_chunk[c], 16)

    with nc.Block("k") as blk:

        @blk.sync
        def _(eng):
            for c in _SYNC_ORDER:
                emit_chunk_dma(eng, c)

        @blk.scalar
        def _(eng):
            for c in _SCAL_ORDER:
                emit_chunk_dma(eng, c)
            for i, t in enumerate(tiles):
                if t["eng"] != "A":
                    continue
                buf = i % 4
                eng.wait_ge(sem_mm, i + 1)
                red = ps.ap()[:, 2 * buf:2 * buf + t["nbank"], 0:t["width"]]
                eng.activation(
                    red, red,
                    mybir.ActivationFunctionType.Abs,
                    accum_out=acc_sb.ap()[:, i:i + 1],
                ).then_inc(sem_a, 1)
            # output from THIS engine: its own accumulator-column writes
            # (ACTIVATE -> READ_ACCUMULATOR) are ordered by program order;
            # DVE columns are fenced by sem_v (tensor_reduce incs after its
            # write).  No completion semaphore: the NEFF postamble outlasts
            # the DMA drain.
            eng.wait_ge(sem_v, nV)
            eng.dma_start(acc_dram.ap()[:, :], acc_sb.ap()[:, :]).then_inc(
                sem_out, 16)
            if keep_wait:
                eng.wait_ge(sem_out, 16)

        @blk.tensor
        def _(eng):
            waited = set()
            for i, t in enumerate(tiles):
                for c in chunk_order:
                    if c in t["chunks"] and c not in waited:
                        waited.add(c)
                        eng.wait_ge(sem_chunk[c], 16)
                if i >= 4:
                    p = i - 4  # previous occupant of this 2-bank buffer
                    eng.wait_ge(sem_v if tiles[p]["eng"] == "V" else sem_a,
                                jobidx[p])
                buf = i % 4
                last = len(t["mms"]) - 1
                for j, (q, coff, n, bank, c0) in enumerate(t["mms"]):
                    ins = nc.tensor.matmul(
                        ps.ap()[:, 2 * buf + bank, c0:c0 + n],
                        a_sb.ap()[32 * q:32 * q + 4, t["acol"]:t["acol"] + P],
                        b_sb.ap()[32 * q:32 * q + 4, coff:coff + n],
                        start=True,
                        stop=True,
                        tile_position=(32 * q, 0),
                    )
                    if j == last:
                        ins.then_inc(sem_mm, 1)

        @blk.vector
        def _(eng):
            for i, t in enumerate(tiles):
                if t["eng"] != "V":
                    continue
                buf = i % 4
                eng.wait_ge(sem_mm, i + 1)
                eng.tensor_reduce(
                    acc_sb.ap()[:, i:i + 1],
                    ps.ap()[:, 2 * buf:2 * buf + t["nbank"], 0:t["width"]],
                    axis=mybir.AxisListType.XY, op=mybir.AluOpType.add,
                    apply_absolute_value=True,
                ).then_inc(sem_v, 1)

    nc.compile()
    return nc


def _get_nc(plan):
    key = ("nc", plan["nb"])
    if key not in _cache:
        _cache[key] = _build(plan)
    return _cache[key]


def _prepare(pred, gt, ift, imf):
    """Compact + pad + build per-core input maps."""
    import ml_dtypes

    p_full = np.asarray(pred).astype(np.float32)
    gt = np.asarray(gt).astype(np.float32)
    t_full = gt[:, ift]
    f_full = gt[:, imf] == 1
    idx = np.flatnonzero(f_full)
    n1 = len(idx)

    plan = _plan(n1)
    npad = plan["nb"] * P
    p = np.zeros(npad, np.float32)
    t = np.zeros(npad, np.float32)
    w = np.zeros(npad, np.float32)
    p[:n1] = p_full[idx]
    t[:n1] = t_full[idx]
    w[:n1] = 1.0
    u = p * t

    # compaction makes the flags trivial: real entries are all flagged,
    # padded entries are exactly zero in every factor.
    A = np.ascontiguousarray(
        np.stack([u, w, p, t]).astype(ml_dtypes.bfloat16)
    )
    Bm = np.ascontiguousarray(
        np.stack([w, u, -t, -p]).astype(ml_dtypes.bfloat16)
    )
    Bh = Bm * np.asarray(0.5, dtype=ml_dtypes.bfloat16)  # exact halving

    nbc, eh, mainw = plan["nbc"], plan["eh"], plan["mainw"]
    gangs = plan["gangs"]
    awidth = P * len(gangs)
    in_maps = []
    for k in range(NCORE):
        a_rows = np.zeros((P, awidth), dtype=ml_dtypes.bfloat16)
        for g, sz in enumerate(gangs):
            off = sum(gangs[:g])
            for q in range(sz):
                blk = nbc * k + off + q
                a_rows[32 * q:32 * q + 4, P * g:P * g + P] = \
                    A[:, P * blk:P * blk + P]

        b_colsk = np.empty((4, plan["bcols"]), dtype=ml_dtypes.bfloat16)
        cols = (P * (nbc * k + 1) + np.arange(mainw)) % npad
        b_colsk[:, 0:mainw] = Bm[:, cols]
        for a in range(nbc):
            c0 = (P * (nbc * k + a) + np.arange(P)) % npad
            ch = (P * (nbc * k + a + eh) + np.arange(P)) % npad
            base = mainw + 256 * a
            b_colsk[:, base:base + P] = Bh[:, c0]
            b_colsk[:, base + P:base + 256] = Bh[:, ch]
        in_maps.append(
            {"a_rows": a_rows, "b_cols": np.ascontiguousarray(b_colsk)}
        )
    return in_maps, A, Bm, n1, plan


def kernel(pred, gt, gt_fracTime, gt_ifMOF):
    from concourse import bass_utils

    ift = int(np.asarray(gt_fracTime))
    imf = int(np.asarray(gt_ifMOF))

    in_maps, A, Bm, n1, plan = _prepare(pred, gt, ift, imf)
    nc = _get_nc(plan)
    res = bass_utils.run_bass_kernel_spmd(nc, in_maps,
                                          core_ids=list(range(NCORE)))

    # T = sum_{i<j} |M| (all device accumulator columns are weight 1)
    T = 0.0
    for r in res.results:
        T += r["acc"].astype(np.float64).sum()

    # host closed form in fp64 over the same bf16 values the device used:
    # sum_{i<j} M = (sum_{i,j} M - sum_diag M) / 2
    A64 = A.astype(np.float64)
    B64 = Bm.astype(np.float64)
    S_all = (A64.sum(axis=1) * B64.sum(axis=1)).sum()
    D_diag = (A64 * B64).sum()
    S_half = (S_all - D_diag) / 2.0

    n_pairs = (float(n1) * float(n1) - float(n1)) / 2.0

    loss = 0.5 * (S_half + T) / 100.0 / n_pairs
    return np.asarray(np.float32(loss))
